# revision 7
# baseline (speedup 1.0000x reference)
"""Cross-attention Trainium2 kernel (Bass/Tile), SPMD over 8 NeuronCores.

Reference computation (per batch element b):
    xs = x[b].reshape(C, H*W).T            # (N, C)   N=4096 tokens
    q  = xs @ Wq + bq                      # (N, C)
    k  = ctx[b] @ Wk + bk                  # (T, C)   T=77
    v  = ctx[b] @ Wv + bv                  # (T, C)
    per head i (d=80): s_i = q_i k_i^T * d^-0.5 ; a_i = softmax(s_i)
    out = concat_i(a_i v_i) @ Wo + bo      # (N, C)
    y[b] = out.T.reshape(C, H, W)

Sharding: data-parallel over batch, 2 images per core.

On-chip layout is "transposed" (channels on partitions, tokens on the free
axis) so x needs no transpose.  v2 design notes (vs the v1 baseline):
  - q, K^T, exp(scores) and V are bf16 on SBUF: halves ScalarE/VectorE
    PSUM-drain time (the v1 bottleneck: S+V were ~30us/block vs PE 16us)
    while all matmuls still accumulate fp32 in PSUM.  The attention
    output path (onhall/outT/outn/Wo/Wq/x) stays fp32/f32r.
  - attnV output (97,512) is drained in ONE copy per head into a single
    staging tile onhall (97, 8*512); row 96 is the softmax denominator
    (ones-column trick), so no separate per-head r extraction: the r row
    of all heads is onhall[96:97,:], reshaped via one DMA to (32,128) so
    the reciprocal runs on 32 lanes instead of 8 (v1: 3.3us -> ~0.4us).
  - x load and y store are ONE dma_start per block with a 3D access
    pattern (v1: 5 each); Sync-engine DIRECT2D dispatch (~0.55us each)
    was near 50% occupancy in v1.
  - K/V projections for BOTH images run once at prep with the token axis
    of the two images concatenated and zero-padded to 256 so the f32r
    matmuls hit full rate (v1: N=77 fp32 quarter-rate matmuls).
  - 1/r is broadcast to (128,512) via a selection-matrix matmul (PE) and
    applied with one tensor_mul per 128-channel tile, as in v1.
"""

import numpy as np

# ---- problem constants (hardcoded per contest contract) ----
B, C, HH, WW = 16, 640, 64, 64
NTOK = HH * WW          # 4096
T = 77
T2 = 2 * T              # both images' tokens, concatenated
TP = 256                # zero-padded token axis for full-rate f32r matmuls
CTX = 768
NH = 8
D = C // NH             # 80
SCALE = float(D) ** -0.5
N_CORES = 8
BPC = B // N_CORES      # 2 images per core
BLK = 512
NBLK = NTOK // BLK      # 8
KC = C // 128           # 5
KX = CTX // 128         # 6
RROW = 96               # denominator row in the attnV psum (legal start)

# head i covers channel rows [80i, 80i+80); split at 128-tile edges:
# (head, tile, lo, hi, dlo): rows [lo,hi) of tile `tile` = head dims
# [dlo, dlo+hi-lo)
PIECES = []
for _i in range(NH):
    _c0, _c1 = D * _i, D * (_i + 1)
    for _t in range(_c0 // 128, (_c1 - 1) // 128 + 1):
        _lo, _hi = max(_c0, 128 * _t), min(_c1, 128 * (_t + 1))
        PIECES.append((_i, _t, _lo - 128 * _t, _hi - 128 * _t, _lo - _c0))

_CACHE = {}


def _build_nc():
    from contextlib import ExitStack
    import concourse.bacc as bacc
    import concourse.tile as tile
    import concourse.mybir as mybir
    from concourse.masks import make_identity

    f32 = mybir.dt.float32
    f32r = mybir.dt.float32r
    bf16 = mybir.dt.bfloat16
    AF = mybir.ActivationFunctionType

    nc = bacc.Bacc("TRN2", target_bir_lowering=False, debug=False,
                   num_devices=N_CORES)

    x_d = nc.dram_tensor("x", [BPC, C, NTOK], f32, kind="ExternalInput").ap()
    ctx_d = nc.dram_tensor("ctxt", [BPC, T, CTX], f32, kind="ExternalInput").ap()
    wq_d = nc.dram_tensor("wq", [C, C], f32, kind="ExternalInput").ap()
    wk_d = nc.dram_tensor("wk", [CTX, C], f32, kind="ExternalInput").ap()
    wv_d = nc.dram_tensor("wv", [CTX, C], f32, kind="ExternalInput").ap()
    wo_d = nc.dram_tensor("wo", [C, C], f32, kind="ExternalInput").ap()
    bq_d = nc.dram_tensor("bq", [C], f32, kind="ExternalInput").ap()
    bk_d = nc.dram_tensor("bk", [C], f32, kind="ExternalInput").ap()
    bv_d = nc.dram_tensor("bv", [C], f32, kind="ExternalInput").ap()
    bo_d = nc.dram_tensor("bo", [C], f32, kind="ExternalInput").ap()
    out_d = nc.dram_tensor("out", [BPC, C, NTOK], f32, kind="ExternalOutput").ap()

    with tile.TileContext(nc) as tc, ExitStack() as ctx:
        wpool = ctx.enter_context(tc.tile_pool(name="wpool", bufs=1))
        cpool = ctx.enter_context(tc.tile_pool(name="cpool", bufs=1))
        ipool = ctx.enter_context(tc.tile_pool(name="ipool", bufs=1))
        xpool = ctx.enter_context(tc.tile_pool(name="xpool", bufs=2))
        qpool = ctx.enter_context(tc.tile_pool(name="qpool", bufs=2))
        epool = ctx.enter_context(tc.tile_pool(name="epool", bufs=2))
        hpool = ctx.enter_context(tc.tile_pool(name="hpool", bufs=2))
        opool = ctx.enter_context(tc.tile_pool(name="opool", bufs=2))
        fpool = ctx.enter_context(tc.tile_pool(name="fpool", bufs=2))
        # PSUM: 8 banks total.  qproj/oproj share ppa (3), scores psr (2),
        # attnV pso (2), R-broadcast prr (1).
        ppa = ctx.enter_context(tc.tile_pool(name="ppa", bufs=3, space="PSUM"))
        psr = ctx.enter_context(tc.tile_pool(name="psr", bufs=2, space="PSUM"))
        pso = ctx.enter_context(tc.tile_pool(name="pso", bufs=2, space="PSUM"))
        prr = ctx.enter_context(tc.tile_pool(name="prr", bufs=1, space="PSUM"))

        # ---------- persistent weights / constants ----------
        def load_w(dram, n_k, tag):
            ts = []
            for k in range(n_k):
                t = wpool.tile([128, C], f32r, tag=f"{tag}{k}")
                nc.sync.dma_start(t, dram[128 * k:128 * (k + 1), :].bitcast(f32r))
                ts.append(t)
            return ts

        wq_sb = load_w(wq_d, KC, "wq")
        wo_sb = load_w(wo_d, KC, "wo")
        wk_sb = load_w(wk_d, KX, "wk")
        wv_sb = load_w(wv_d, KX, "wv")
        bv_row = wpool.tile([1, C], f32r, tag="bvrow")
        nc.sync.dma_start(bv_row, bv_d[None, :].bitcast(f32r))

        # per-partition bias layouts: bias[128m + p] at [p, m]
        bqT = wpool.tile([128, KC], f32, tag="bqT")
        nc.sync.dma_start(bqT, bq_d.rearrange("(m p) -> p m", p=128))
        boT = wpool.tile([128, KC], f32, tag="boT")
        nc.sync.dma_start(boT, bo_d.rearrange("(m p) -> p m", p=128))
        bkT = wpool.tile([128, KC], f32, tag="bkT")
        nc.sync.dma_start(bkT, bk_d.rearrange("(m p) -> p m", p=128))
        bkTs = wpool.tile([128, KC], f32, tag="bkTs")
        nc.vector.tensor_scalar_mul(bkTs, bkT, SCALE)

        ident = cpool.tile([128, 128], f32, tag="ident")
        make_identity(nc, ident)
        zeros32 = cpool.tile([128, 128], f32, tag="zeros32")
        nc.vector.memset(zeros32, 0.0)
        ones32 = cpool.tile([128, 1], f32, tag="ones32")
        nc.vector.memset(ones32, 1.0)
        ones_row32 = cpool.tile([1, 128], f32, tag="onesrow32")
        nc.vector.memset(ones_row32, 1.0)
        ones77 = cpool.tile([1, T], f32r, tag="ones77")
        nc.vector.tensor_copy(ones77, ones_row32[:, 0:T])

        # selection matrices S_t (8,128): S_t[i,p] = 1 iff head(128t+p) == i
        S_sel = []
        for t in range(KC):
            s32 = cpool.tile([NH, 128], f32, tag=f"s32_{t}")
            nc.gpsimd.memset(s32, 1.0)
            # cond A: p + 128t - 80i >= 0
            nc.gpsimd.affine_select(
                out=s32, in_=s32, compare_op=mybir.AluOpType.is_ge, fill=0.0,
                base=128 * t, pattern=[[1, 128]], channel_multiplier=-D)
            # cond B: -p - 128t + 80i + 79 >= 0
            nc.gpsimd.affine_select(
                out=s32, in_=s32, compare_op=mybir.AluOpType.is_ge, fill=0.0,
                base=D - 1 - 128 * t, pattern=[[-1, 128]], channel_multiplier=D)
            st = cpool.tile([NH, 128], f32r, tag=f"ssel_{t}")
            nc.vector.tensor_copy(st, s32)
            S_sel.append(st)

        # ---------- per-pair prep: K/V for both images in one pass ----------
        # ctx^T tiles (128, TP) f32r: cols [77b:77b+77] = image b, rest zero.
        ctx_sb = []
        for b in range(BPC):
            t = ipool.tile([T, CTX], f32, tag=f"ctx{b}")
            nc.sync.dma_start(t, ctx_d[b])
            ctx_sb.append(t)

        ctxT = []
        for k in range(KX):
            t = ipool.tile([128, TP], f32r, tag=f"ctxT{k}")
            nc.vector.tensor_copy(t[:, T2:TP], zeros32[:, 0:TP - T2])
            ctxT.append(t)
        for b in range(BPC):
            for k in range(KX):
                pt = psr.tile([128, T], f32, tag="psr")
                nc.tensor.transpose(pt, ctx_sb[b][:, 128 * k:128 * (k + 1)],
                                    ident[0:T, 0:T])
                nc.vector.tensor_copy(
                    ctxT[k][:, T * b:T * (b + 1)].bitcast(f32r), pt)

        # K^T = scale * (Wk^T @ ctx^T + bk), both images at once, bf16 out
        ktmp = []
        for m in range(KC):
            pt = pso.tile([128, TP], f32, tag="pso")
            for k in range(KX):
                nc.tensor.matmul(
                    pt, wk_sb[k][:, 128 * m:128 * (m + 1)], ctxT[k],
                    start=(k == 0), stop=(k == KX - 1))
            t = ipool.tile([128, T2], bf16, tag=f"ktmp{m}")
            nc.scalar.activation(t, pt[:, 0:T2], AF.Identity,
                                 bias=bkTs[:, m:m + 1], scale=SCALE)
            ktmp.append(t)

        # zero-padded per-(image, head, tile) lhsT pieces for scores
        kTp = {}
        for b in range(BPC):
            for (i, tt, lo, hi, dlo) in PIECES:
                t = ipool.tile([128, T], bf16, tag=f"kTp{b}_{i}_{tt}")
                nc.vector.tensor_copy(t, zeros32[:, 0:T])
                nc.sync.dma_start(t[lo:hi, :],
                                  ktmp[tt][lo:hi, T * b:T * (b + 1)])
                kTp[(b, i, tt)] = t

        # V heads: vA_i = [V_i | zeros | ones]  (77, 97) bf16, ones col at 96
        vA = {}
        for b in range(BPC):
            for i in range(NH):
                t = ipool.tile([T, RROW + 1], bf16, tag=f"vA{b}_{i}")
                nc.vector.tensor_copy(t[:, D:RROW], zeros32[0:T, 0:RROW - D])
                nc.vector.tensor_copy(t[:, RROW:RROW + 1], ones32[0:T, :])
                vA[(b, i)] = t
            for h2 in range(2):
                pt = pso.tile([T, 320], f32, tag="pso")
                for k in range(KX):
                    nc.tensor.matmul(
                        pt, ctxT[k][:, T * b:T * (b + 1)].bitcast(f32r),
                        wv_sb[k][:, 320 * h2:320 * (h2 + 1)],
                        start=(k == 0), stop=False)
                nc.tensor.matmul(
                    pt, ones77, bv_row[:, 320 * h2:320 * (h2 + 1)],
                    start=False, stop=True)
                for i in range(4 * h2, 4 * h2 + 4):
                    off = D * i - 320 * h2
                    nc.vector.tensor_copy(vA[(b, i)][:, 0:D],
                                          pt[:, off:off + D])

        # ---------- 512-token blocks ----------
        def x_prefetch(n):
            b, c0 = n // NBLK, BLK * (n % NBLK)
            t = xpool.tile([128, KC, BLK], f32r, tag="xall")
            src = x_d[b].rearrange("(k p) n -> p k n", p=128)
            nc.sync.dma_start(t, src[:, :, c0:c0 + BLK].bitcast(f32r))
            return t

        xtiles = {0: x_prefetch(0)}

        for n in range(BPC * NBLK):
            b, c0 = n // NBLK, BLK * (n % NBLK)
            if n + 1 < BPC * NBLK:
                xtiles[n + 1] = x_prefetch(n + 1)
            xall = xtiles.pop(n)

            # qT = Wq^T @ x (+bq), 5 m-tiles of (128, 512), bf16 out
            qtmp = []
            for m in range(KC):
                pt = ppa.tile([128, BLK], f32, tag="ppa")
                for k in range(KC):
                    nc.tensor.matmul(
                        pt, wq_sb[k][:, 128 * m:128 * (m + 1)], xall[:, k, :],
                        start=(k == 0), stop=(k == KC - 1))
                t = qpool.tile([128, BLK], bf16, tag=f"q{m}")
                nc.scalar.activation(t, pt, AF.Identity,
                                     bias=bqT[:, m:m + 1])
                qtmp.append(t)

            # per-head attention; onhall row 96 = softmax denominator
            onhall = hpool.tile([RROW + 1, NH * BLK], bf16, tag="onhall")
            outT = opool.tile([128, KC * BLK], bf16, tag="outT")
            for i in range(NH):
                spans = [p for p in PIECES if p[0] == i]
                sps = psr.tile([T, BLK], f32, tag="psr")
                for j, (_, tt, _, _, _) in enumerate(spans):
                    nc.tensor.matmul(
                        sps, kTp[(b, i, tt)], qtmp[tt],
                        start=(j == 0), stop=(j == len(spans) - 1))
                e_sb = epool.tile([T, BLK], bf16, tag=f"e{i % 2}")
                nc.scalar.activation(e_sb, sps, AF.Exp)
                opt = pso.tile([RROW + 1, BLK], f32, tag="pso")
                nc.tensor.matmul(opt, vA[(b, i)], e_sb, start=True, stop=True)
                nc.vector.tensor_copy(
                    onhall[:, BLK * i:BLK * (i + 1)], opt)
                # scatter head rows into channel-major outT (SBUF->SBUF DMA)
                for (_, tt, lo, hi, dlo) in spans:
                    nc.sync.dma_start(
                        outT[lo:hi, BLK * tt:BLK * tt + BLK],
                        onhall[dlo:dlo + hi - lo, BLK * i:BLK * (i + 1)])

            # softmax denominators: (1, 8*512) -> (32,128) -> recip ->
            # rall (8,512) f32r for the broadcast matmul
            r32 = hpool.tile([32, 128], bf16, tag="r32")
            r32f = hpool.tile([32, 128], f32, tag="r32f")
            nc.sync.dma_start(r32, onhall[RROW:RROW + 1, :])
            nc.vector.reciprocal(r32f, r32)
            rall = hpool.tile([NH, BLK], f32r, tag="rall")
            nc.sync.dma_start(rall, r32f.bitcast(f32r))

            # normalize: outn_t = outT_t * (S_t^T @ rall)
            outn = opool.tile([128, KC * BLK], f32r, tag="outn")
            for tt in range(KC):
                Rp = prr.tile([128, BLK], f32, tag="prr")
                nc.tensor.matmul(Rp, S_sel[tt], rall, start=True, stop=True)
                nc.vector.tensor_mul(
                    outn[:, BLK * tt:BLK * (tt + 1)],
                    outT[:, BLK * tt:BLK * (tt + 1)], Rp)

            # y = Wo^T @ outn + bo, straight to the output layout
            fin = fpool.tile([128, KC, BLK], f32, tag="fin")
            for m in range(KC):
                pt = ppa.tile([128, BLK], f32, tag="ppa")
                for k in range(KC):
                    nc.tensor.matmul(
                        pt, wo_sb[k][:, 128 * m:128 * (m + 1)],
                        outn[:, BLK * k:BLK * (k + 1)],
                        start=(k == 0), stop=(k == KC - 1))
                nc.scalar.activation(fin[:, m, :], pt, AF.Identity,
                                     bias=boT[:, m:m + 1])
            dst = out_d[b].rearrange("(k p) n -> p k n", p=128)
            nc.sync.dma_start(dst[:, :, c0:c0 + BLK], fin)
    nc.compile()
    return nc


def _get_nc():
    if "nc" not in _CACHE:
        _CACHE["nc"] = _build_nc()
    return _CACHE["nc"]


def kernel(**inputs):
    from concourse.bass_utils import run_bass_kernel_spmd

    x = np.asarray(inputs["x"], dtype=np.float32)
    context = np.asarray(inputs["context"], dtype=np.float32)
    wq = np.ascontiguousarray(np.asarray(inputs["Wq"], dtype=np.float32))
    wk = np.ascontiguousarray(np.asarray(inputs["Wk"], dtype=np.float32))
    wv = np.ascontiguousarray(np.asarray(inputs["Wv"], dtype=np.float32))
    wo = np.ascontiguousarray(np.asarray(inputs["Wo"], dtype=np.float32))
    bq = np.ascontiguousarray(np.asarray(inputs["bq"], dtype=np.float32))
    bk = np.ascontiguousarray(np.asarray(inputs["bk"], dtype=np.float32))
    bv = np.ascontiguousarray(np.asarray(inputs["bv"], dtype=np.float32))
    bo = np.ascontiguousarray(np.asarray(inputs["bo"], dtype=np.float32))

    xs = np.ascontiguousarray(x.reshape(B, C, NTOK))
    ctxs = np.ascontiguousarray(context)

    nc = _get_nc()
    in_maps = []
    for c in range(N_CORES):
        sl = slice(BPC * c, BPC * (c + 1))
        in_maps.append({
            "x": np.ascontiguousarray(xs[sl]),
            "ctxt": np.ascontiguousarray(ctxs[sl]),
            "wq": wq, "wk": wk, "wv": wv, "wo": wo,
            "bq": bq, "bk": bk, "bv": bv, "bo": bo,
        })
    res = run_bass_kernel_spmd(nc, in_maps, list(range(N_CORES))).results
    out = np.concatenate([res[c]["out"] for c in range(N_CORES)], axis=0)
    return np.ascontiguousarray(out.reshape(B, C, HH, WW))


# revision 9
# speedup vs baseline: 1.7707x; 1.7707x over previous
"""Cross-attention Trainium2 kernel (Bass/Tile), SPMD over 8 NeuronCores.

Reference computation (per batch element b):
    xs = x[b].reshape(C, H*W).T            # (N, C)   N=4096 tokens
    q  = xs @ Wq + bq                      # (N, C)
    k  = ctx[b] @ Wk + bk                  # (T, C)   T=77
    v  = ctx[b] @ Wv + bv                  # (T, C)
    per head i (d=80): s_i = q_i k_i^T * d^-0.5 ; a_i = softmax(s_i)
    out = concat_i(a_i v_i) @ Wo + bo      # (N, C)
    y[b] = out.T.reshape(C, H, W)

Sharding: data-parallel over batch, 2 images per core.

On-chip layout is "transposed" (channels on partitions, tokens on the free
axis) so x needs no transpose.  v2 design notes (vs the v1 baseline):
  - q, K^T, exp(scores) and V are bf16 on SBUF: halves ScalarE/VectorE
    PSUM-drain time (the v1 bottleneck: S+V were ~30us/block vs PE 16us)
    while all matmuls still accumulate fp32 in PSUM.  The attention
    output path (onhall/outT/outn/Wo/Wq/x) stays fp32/f32r.
  - attnV output (97,512) is drained in ONE copy per head into a single
    staging tile onhall (97, 8*512); row 96 is the softmax denominator
    (ones-column trick), so no separate per-head r extraction: the r row
    of all heads is onhall[96:97,:], reshaped via one DMA to (32,128) so
    the reciprocal runs on 32 lanes instead of 8 (v1: 3.3us -> ~0.4us).
  - x load and y store are ONE dma_start per block with a 3D access
    pattern (v1: 5 each); Sync-engine DIRECT2D dispatch (~0.55us each)
    was near 50% occupancy in v1.
  - K/V projections for BOTH images run once at prep with the token axis
    of the two images concatenated and zero-padded to 256 so the f32r
    matmuls hit full rate (v1: N=77 fp32 quarter-rate matmuls).
  - 1/r is broadcast to (128,512) via a selection-matrix matmul (PE) and
    applied with one tensor_mul per 128-channel tile, as in v1.
"""

import numpy as np

# ---- problem constants (hardcoded per contest contract) ----
B, C, HH, WW = 16, 640, 64, 64
NTOK = HH * WW          # 4096
T = 77
T2 = 2 * T              # both images' tokens, concatenated
TP = 256                # zero-padded token axis for full-rate f32r matmuls
CTX = 768
NH = 8
D = C // NH             # 80
SCALE = float(D) ** -0.5
N_CORES = 8
BPC = B // N_CORES      # 2 images per core
BLK = 512
NBLK = NTOK // BLK      # 8
KC = C // 128           # 5
KX = CTX // 128         # 6
RROW = 96               # denominator row in the attnV psum (legal start)

# head i covers channel rows [80i, 80i+80); split at 128-tile edges:
# (head, tile, lo, hi, dlo): rows [lo,hi) of tile `tile` = head dims
# [dlo, dlo+hi-lo)
PIECES = []
for _i in range(NH):
    _c0, _c1 = D * _i, D * (_i + 1)
    for _t in range(_c0 // 128, (_c1 - 1) // 128 + 1):
        _lo, _hi = max(_c0, 128 * _t), min(_c1, 128 * (_t + 1))
        PIECES.append((_i, _t, _lo - 128 * _t, _hi - 128 * _t, _lo - _c0))

_CACHE = {}


def _build_nc():
    from contextlib import ExitStack
    import concourse.bacc as bacc
    import concourse.tile as tile
    import concourse.mybir as mybir
    from concourse.masks import make_identity

    f32 = mybir.dt.float32
    f32r = mybir.dt.float32r
    bf16 = mybir.dt.bfloat16
    AF = mybir.ActivationFunctionType

    nc = bacc.Bacc("TRN2", target_bir_lowering=False, debug=False,
                   num_devices=N_CORES)

    x_d = nc.dram_tensor("x", [BPC, C, NTOK], f32, kind="ExternalInput").ap()
    ctx_d = nc.dram_tensor("ctxt", [BPC, T, CTX], f32, kind="ExternalInput").ap()
    wq_d = nc.dram_tensor("wq", [C, C], f32, kind="ExternalInput").ap()
    wk_d = nc.dram_tensor("wk", [CTX, C], f32, kind="ExternalInput").ap()
    wv_d = nc.dram_tensor("wv", [CTX, C], f32, kind="ExternalInput").ap()
    wo_d = nc.dram_tensor("wo", [C, C], f32, kind="ExternalInput").ap()
    bq_d = nc.dram_tensor("bq", [C], f32, kind="ExternalInput").ap()
    bk_d = nc.dram_tensor("bk", [C], f32, kind="ExternalInput").ap()
    bv_d = nc.dram_tensor("bv", [C], f32, kind="ExternalInput").ap()
    bo_d = nc.dram_tensor("bo", [C], f32, kind="ExternalInput").ap()
    out_d = nc.dram_tensor("out", [BPC, C, NTOK], f32, kind="ExternalOutput").ap()

    with tile.TileContext(nc) as tc, ExitStack() as ctx:
        wpool = ctx.enter_context(tc.tile_pool(name="wpool", bufs=1))
        cpool = ctx.enter_context(tc.tile_pool(name="cpool", bufs=1))
        ipool = ctx.enter_context(tc.tile_pool(name="ipool", bufs=1))
        xpool = ctx.enter_context(tc.tile_pool(name="xpool", bufs=2))
        qpool = ctx.enter_context(tc.tile_pool(name="qpool", bufs=2))
        epool = ctx.enter_context(tc.tile_pool(name="epool", bufs=2))
        hpool = ctx.enter_context(tc.tile_pool(name="hpool", bufs=2))
        opool = ctx.enter_context(tc.tile_pool(name="opool", bufs=2))
        fpool = ctx.enter_context(tc.tile_pool(name="fpool", bufs=2))
        # PSUM: 8 banks total.  qproj/oproj share ppa (2), scores psr (2),
        # attnV pso (2), R-broadcast prr (2).
        ppa = ctx.enter_context(tc.tile_pool(name="ppa", bufs=2, space="PSUM"))
        psr = ctx.enter_context(tc.tile_pool(name="psr", bufs=2, space="PSUM"))
        pso = ctx.enter_context(tc.tile_pool(name="pso", bufs=2, space="PSUM"))
        prr = ctx.enter_context(tc.tile_pool(name="prr", bufs=2, space="PSUM"))

        # ---------- persistent weights / constants ----------
        def load_w(dram, n_k, tag):
            ts = []
            for k in range(n_k):
                t = wpool.tile([128, C], f32r, tag=f"{tag}{k}")
                nc.sync.dma_start(t, dram[128 * k:128 * (k + 1), :].bitcast(f32r))
                ts.append(t)
            return ts

        wq_sb = load_w(wq_d, KC, "wq")
        wo_sb = load_w(wo_d, KC, "wo")
        wk_sb = load_w(wk_d, KX, "wk")
        wv_sb = load_w(wv_d, KX, "wv")
        bv_row = wpool.tile([1, C], f32r, tag="bvrow")
        nc.sync.dma_start(bv_row, bv_d[None, :].bitcast(f32r))

        # per-partition bias layouts: bias[128m + p] at [p, m]
        bqT = wpool.tile([128, KC], f32, tag="bqT")
        nc.sync.dma_start(bqT, bq_d.rearrange("(m p) -> p m", p=128))
        boT = wpool.tile([128, KC], f32, tag="boT")
        nc.sync.dma_start(boT, bo_d.rearrange("(m p) -> p m", p=128))
        bkT = wpool.tile([128, KC], f32, tag="bkT")
        nc.sync.dma_start(bkT, bk_d.rearrange("(m p) -> p m", p=128))
        bkTs = wpool.tile([128, KC], f32, tag="bkTs")
        nc.vector.tensor_scalar_mul(bkTs, bkT, SCALE)

        ident = cpool.tile([128, 128], f32, tag="ident")
        make_identity(nc, ident)
        zeros32 = cpool.tile([128, 128], f32, tag="zeros32")
        nc.vector.memset(zeros32, 0.0)
        ones32 = cpool.tile([128, 1], f32, tag="ones32")
        nc.vector.memset(ones32, 1.0)
        ones_row32 = cpool.tile([1, 128], f32, tag="onesrow32")
        nc.vector.memset(ones_row32, 1.0)
        ones77 = cpool.tile([1, T], f32r, tag="ones77")
        nc.vector.tensor_copy(ones77, ones_row32[:, 0:T])

        # selection matrices S_t (8,128): S_t[i,p] = 1 iff head(128t+p) == i
        S_sel = []
        for t in range(KC):
            s32 = cpool.tile([NH, 128], f32, tag=f"s32_{t}")
            nc.gpsimd.memset(s32, 1.0)
            # cond A: p + 128t - 80i >= 0
            nc.gpsimd.affine_select(
                out=s32, in_=s32, compare_op=mybir.AluOpType.is_ge, fill=0.0,
                base=128 * t, pattern=[[1, 128]], channel_multiplier=-D)
            # cond B: -p - 128t + 80i + 79 >= 0
            nc.gpsimd.affine_select(
                out=s32, in_=s32, compare_op=mybir.AluOpType.is_ge, fill=0.0,
                base=D - 1 - 128 * t, pattern=[[-1, 128]], channel_multiplier=D)
            st = cpool.tile([NH, 128], f32r, tag=f"ssel_{t}")
            nc.vector.tensor_copy(st, s32)
            S_sel.append(st)

        # ---------- per-pair prep: K/V for both images in one pass ----------
        # ctx^T tiles (128, TP) f32r: cols [77b:77b+77] = image b, rest zero.
        ctx_sb = []
        for b in range(BPC):
            t = ipool.tile([T, CTX], f32, tag=f"ctx{b}")
            nc.sync.dma_start(t, ctx_d[b])
            ctx_sb.append(t)

        ctxT = []
        for k in range(KX):
            t = ipool.tile([128, TP], f32r, tag=f"ctxT{k}")
            nc.vector.tensor_copy(t[:, T2:TP], zeros32[:, 0:TP - T2])
            ctxT.append(t)
        for b in range(BPC):
            for k in range(KX):
                pt = psr.tile([128, T], f32, tag="psr")
                nc.tensor.transpose(pt, ctx_sb[b][:, 128 * k:128 * (k + 1)],
                                    ident[0:T, 0:T])
                nc.vector.tensor_copy(
                    ctxT[k][:, T * b:T * (b + 1)].bitcast(f32r), pt)

        # K^T = scale * (Wk^T @ ctx^T + bk), both images at once, bf16 out
        ktmp = []
        for m in range(KC):
            pt = pso.tile([128, TP], f32, tag="pso")
            for k in range(KX):
                nc.tensor.matmul(
                    pt, wk_sb[k][:, 128 * m:128 * (m + 1)], ctxT[k],
                    start=(k == 0), stop=(k == KX - 1))
            t = ipool.tile([128, T2], bf16, tag=f"ktmp{m}")
            nc.scalar.activation(t, pt[:, 0:T2], AF.Identity,
                                 bias=bkTs[:, m:m + 1], scale=SCALE)
            ktmp.append(t)

        # zero-padded per-(image, head, tile) lhsT pieces for scores
        kTp = {}
        for b in range(BPC):
            for (i, tt, lo, hi, dlo) in PIECES:
                t = ipool.tile([128, T], bf16, tag=f"kTp{b}_{i}_{tt}")
                nc.vector.tensor_copy(t, zeros32[:, 0:T])
                nc.sync.dma_start(t[lo:hi, :],
                                  ktmp[tt][lo:hi, T * b:T * (b + 1)])
                kTp[(b, i, tt)] = t

        # V heads: vA_i = [V_i | zeros | ones]  (77, 97) bf16, ones col at 96
        vA = {}
        for b in range(BPC):
            for i in range(NH):
                t = ipool.tile([T, RROW + 1], bf16, tag=f"vA{b}_{i}")
                nc.vector.tensor_copy(t[:, D:RROW], zeros32[0:T, 0:RROW - D])
                nc.vector.tensor_copy(t[:, RROW:RROW + 1], ones32[0:T, :])
                vA[(b, i)] = t
            for h2 in range(2):
                pt = pso.tile([T, 320], f32, tag="pso")
                for k in range(KX):
                    nc.tensor.matmul(
                        pt, ctxT[k][:, T * b:T * (b + 1)].bitcast(f32r),
                        wv_sb[k][:, 320 * h2:320 * (h2 + 1)],
                        start=(k == 0), stop=False)
                nc.tensor.matmul(
                    pt, ones77, bv_row[:, 320 * h2:320 * (h2 + 1)],
                    start=False, stop=True)
                for i in range(4 * h2, 4 * h2 + 4):
                    off = D * i - 320 * h2
                    nc.vector.tensor_copy(vA[(b, i)][:, 0:D],
                                          pt[:, off:off + D])

        # ---------- 512-token blocks ----------
        def x_prefetch(n):
            b, c0 = n // NBLK, BLK * (n % NBLK)
            t = xpool.tile([128, KC, BLK], f32r, tag="xall")
            src = x_d[b].rearrange("(k p) n -> p k n", p=128)
            nc.sync.dma_start(t, src[:, :, c0:c0 + BLK].bitcast(f32r))
            return t

        xtiles = {0: x_prefetch(0)}

        # -- stage 1: qproj + per-head attention + denominator chain --
        def stage1(n):
            b = n // NBLK
            xall = xtiles.pop(n)

            # qT = Wq^T @ x (+bq), 5 m-tiles of (128, 512), bf16 out
            qtmp = []
            for m in range(KC):
                pt = ppa.tile([128, BLK], f32, tag="ppa")
                for k in range(KC):
                    nc.tensor.matmul(
                        pt, wq_sb[k][:, 128 * m:128 * (m + 1)], xall[:, k, :],
                        start=(k == 0), stop=(k == KC - 1))
                t = qpool.tile([128, BLK], bf16, tag=f"q{m}")
                nc.scalar.activation(t, pt, AF.Identity,
                                     bias=bqT[:, m:m + 1])
                qtmp.append(t)

            # per-head attention; onhall row 96 = softmax denominator.
            # Lag attnV one head behind scores so EXP overlaps the next
            # head's scores matmuls in the static PE order.
            onhall = hpool.tile([RROW + 1, NH * BLK], bf16, tag="onhall")
            outT = opool.tile([128, KC * BLK], bf16, tag="outT")
            es = {}

            def do_scores(i):
                spans = [p for p in PIECES if p[0] == i]
                sps = psr.tile([T, BLK], f32, tag="psr")
                for j, (_, tt, _, _, _) in enumerate(spans):
                    nc.tensor.matmul(
                        sps, kTp[(b, i, tt)], qtmp[tt],
                        start=(j == 0), stop=(j == len(spans) - 1))
                e_sb = epool.tile([T, BLK], bf16, tag=f"e{i % 2}")
                nc.scalar.activation(e_sb, sps, AF.Exp)
                es[i] = e_sb

            def do_attnv(i):
                spans = [p for p in PIECES if p[0] == i]
                opt = pso.tile([RROW + 1, BLK], f32, tag="pso")
                nc.tensor.matmul(opt, vA[(b, i)], es.pop(i),
                                 start=True, stop=True)
                nc.vector.tensor_copy(
                    onhall[:, BLK * i:BLK * (i + 1)], opt)
                # scatter head rows into channel-major outT (SBUF->SBUF,
                # dispatched from the idle GpSimd SWDGE queue)
                for (_, tt, lo, hi, dlo) in spans:
                    nc.gpsimd.dma_start(
                        outT[lo:hi, BLK * tt:BLK * tt + BLK],
                        onhall[dlo:dlo + hi - lo, BLK * i:BLK * (i + 1)])

            do_scores(0)
            for i in range(1, NH):
                do_scores(i)
                do_attnv(i - 1)
            do_attnv(NH - 1)

            # softmax denominators: (1, 8*512) -> (32,128) -> recip ->
            # rall (8,512) f32r for the broadcast matmul
            r32 = hpool.tile([32, 128], bf16, tag="r32")
            r32f = hpool.tile([32, 128], f32, tag="r32f")
            nc.sync.dma_start(r32, onhall[RROW:RROW + 1, :])
            nc.vector.reciprocal(r32f, r32)
            rall = hpool.tile([NH, BLK], f32r, tag="rall")
            nc.sync.dma_start(rall, r32f.bitcast(f32r))
            return outT, rall

        # -- stage 2: normalize + output projection + store --
        def stage2(n, outT, rall):
            b, c0 = n // NBLK, BLK * (n % NBLK)
            # normalize: outn_t = outT_t * (S_t^T @ rall)
            outn = opool.tile([128, KC * BLK], f32r, tag="outn")
            for tt in range(KC):
                Rp = prr.tile([128, BLK], f32, tag="prr")
                nc.tensor.matmul(Rp, S_sel[tt], rall, start=True, stop=True)
                nc.vector.tensor_mul(
                    outn[:, BLK * tt:BLK * (tt + 1)],
                    outT[:, BLK * tt:BLK * (tt + 1)], Rp)

            # y = Wo^T @ outn + bo, straight to the output layout
            fin = fpool.tile([128, KC, BLK], f32, tag="fin")
            for m in range(KC):
                pt = ppa.tile([128, BLK], f32, tag="ppa")
                for k in range(KC):
                    nc.tensor.matmul(
                        pt, wo_sb[k][:, 128 * m:128 * (m + 1)],
                        outn[:, BLK * k:BLK * (k + 1)],
                        start=(k == 0), stop=(k == KC - 1))
                nc.scalar.activation(fin[:, m, :], pt, AF.Identity,
                                     bias=boT[:, m:m + 1])
            dst = out_d[b].rearrange("(k p) n -> p k n", p=128)
            nc.sync.dma_start(dst[:, :, c0:c0 + BLK], fin)

        # one-block software pipeline: stage2(n-1) is emitted after
        # qproj/heads(n) so the PE covers the denominator-chain latency
        # of block n-1 with block n's matmuls.
        prev = None
        for n in range(BPC * NBLK):
            if n + 1 < BPC * NBLK:
                xtiles[n + 1] = x_prefetch(n + 1)
            cur = stage1(n)
            if prev is not None:
                stage2(n - 1, *prev)
            prev = cur
        stage2(BPC * NBLK - 1, *prev)
    nc.compile()
    return nc


def _get_nc():
    if "nc" not in _CACHE:
        _CACHE["nc"] = _build_nc()
    return _CACHE["nc"]


def kernel(**inputs):
    from concourse.bass_utils import run_bass_kernel_spmd

    x = np.asarray(inputs["x"], dtype=np.float32)
    context = np.asarray(inputs["context"], dtype=np.float32)
    wq = np.ascontiguousarray(np.asarray(inputs["Wq"], dtype=np.float32))
    wk = np.ascontiguousarray(np.asarray(inputs["Wk"], dtype=np.float32))
    wv = np.ascontiguousarray(np.asarray(inputs["Wv"], dtype=np.float32))
    wo = np.ascontiguousarray(np.asarray(inputs["Wo"], dtype=np.float32))
    bq = np.ascontiguousarray(np.asarray(inputs["bq"], dtype=np.float32))
    bk = np.ascontiguousarray(np.asarray(inputs["bk"], dtype=np.float32))
    bv = np.ascontiguousarray(np.asarray(inputs["bv"], dtype=np.float32))
    bo = np.ascontiguousarray(np.asarray(inputs["bo"], dtype=np.float32))

    xs = np.ascontiguousarray(x.reshape(B, C, NTOK))
    ctxs = np.ascontiguousarray(context)

    nc = _get_nc()
    in_maps = []
    for c in range(N_CORES):
        sl = slice(BPC * c, BPC * (c + 1))
        in_maps.append({
            "x": np.ascontiguousarray(xs[sl]),
            "ctxt": np.ascontiguousarray(ctxs[sl]),
            "wq": wq, "wk": wk, "wv": wv, "wo": wo,
            "bq": bq, "bk": bk, "bv": bv, "bo": bo,
        })
    res = run_bass_kernel_spmd(nc, in_maps, list(range(N_CORES))).results
    out = np.concatenate([res[c]["out"] for c in range(N_CORES)], axis=0)
    return np.ascontiguousarray(out.reshape(B, C, HH, WW))
